# revision 2
# baseline (speedup 1.0000x reference)
"""Trainium2 Bass kernel for MoELayerStacks (moe_routing) — expert-sharded.

Sharding strategy (kernel's choice per problem statement): tokens are
ordered by their routed expert (router argmax computed host-side as part
of the sharding function; it is 0.06% of the model's FLOPs) and split
into 8 equal shards of 4096 tokens. Each core therefore sees tokens for
at most NSEG contiguous experts and runs the expert stack densely for
just those segments:

  l1:   psum1[16s+o, t] = W1[e_s].T @ xT   (16 K-tiles, bf16, 64 rows)
  act:  rsq = min(Square(psum1+b1)*255/256, 1); rlin = clip(psum1+b1,0,1)
  l2:   psum2[32s+o2, t] = W2sq[e_s].T @ rsq + W2lin[e_s].T @ rlin
  l3:   psum3[s, t] = W3[e_s].T @ l2x + raw[16s+15, t]  (+ob via ACT)
  out:  res[s, t] per block -> host selects row seg(t) and unpermutes.

x is packed bf16 on the host (l1 rel err ~3e-3 << 2e-2 gate); everything
else stays f32/f32r. DMA-bound: ~17MB x per core at ~320GB/s.
"""

import os
import sys

import numpy as np

for _p in ("/opt/trn_rl_repo",):
    if _p not in sys.path and os.path.isdir(_p):
        sys.path.insert(0, _p)

L2 = 15
L3 = 32
E = 16
ED = 2048
RD = 128
B = 32768
NCORES = 8
BC = B // NCORES      # tokens per core = 4096
NT = 512              # tokens per block
NB = BC // NT         # blocks per core = 8
KT = ED // 128        # K tiles = 16
SQ_SCALE = 255.0 / 256.0
NSEG = 4              # expert segments per core (static; >= actual max)


# ----------------------------------------------------------------------------
# Host-side routing + packing
# ----------------------------------------------------------------------------

def route_tokens(router_input, router_w, router_b):
    """Sharding function: order tokens by routed expert, split into 8 shards."""
    r = np.asarray(router_input, np.float32)
    w = np.asarray(router_w, np.float32)
    b = np.asarray(router_b, np.float32)
    logits = r @ w.T + b
    eidx = np.argmax(logits, axis=1)
    order = np.argsort(eidx, kind="stable")  # tokens sorted by expert
    return eidx, order


def pack_core_weights(experts, l1_w, l1_b, l2_w, l2_b, out_w, out_b):
    """Pack weights for one core's expert segments (len(experts) <= NSEG)."""
    import ml_dtypes

    f = np.float32
    ns = len(experts)
    assert ns <= NSEG
    # w1t[p, kt, 16s+o] = l1_w[e_s, o, kt*128+p]  (bf16)
    w1t = np.zeros((128, KT, NSEG * 16), f)
    b1 = np.zeros((NSEG * 16, 1), f)
    w2sq = np.zeros((NSEG * 16, NSEG * 32), f)
    w2lin = np.zeros((NSEG * 16, NSEG * 32), f)
    b2 = np.zeros((NSEG * 32, 1), f)
    w3p = np.zeros((NSEG * 32, NSEG), f)
    rawsel = np.zeros((NSEG * 16, NSEG), f)
    ob = np.zeros((NSEG, 1), f)
    for s, e in enumerate(experts):
        wt = l1_w[e].transpose(1, 0).reshape(KT, 128, 16)  # [kt, p, o]
        w1t[:, :, 16 * s:16 * s + 16] = wt.transpose(1, 0, 2)
        b1[16 * s:16 * s + 16, 0] = l1_b[e]
        for t in range(L2):
            w2sq[16 * s + t, 32 * s:32 * s + 32] = l2_w[e, :, t]
            w2lin[16 * s + t, 32 * s:32 * s + 32] = l2_w[e, :, L2 + t]
        b2[32 * s:32 * s + 32, 0] = l2_b[e]
        w3p[32 * s:32 * s + 32, s] = out_w[e, 0, :]
        rawsel[16 * s + 15, s] = 1.0
        ob[s, 0] = out_b[e, 0] + l1_b[e, 15]
    return {
        "w1t": w1t.astype(ml_dtypes.bfloat16),
        "b1": b1, "w2sq": w2sq, "w2lin": w2lin, "b2": b2,
        "w3p": w3p, "rawsel": rawsel, "ob": ob,
    }


def pack_x_shard_bf16(x_shard, nb=NB):
    """(L<=nb*NT, 2048) f32 -> (nb, 128, KT, NT) bf16, zero-padded:
    [b,p,kt,j] = x[b*NT+j, kt*128+p]"""
    import ml_dtypes

    L = x_shard.shape[0]
    if L < nb * NT:
        x_shard = np.concatenate(
            [x_shard, np.zeros((nb * NT - L, ED), np.float32)], axis=0)
    xb = x_shard.reshape(nb, NT, KT, 128).transpose(0, 3, 2, 1)
    return np.ascontiguousarray(xb).astype(ml_dtypes.bfloat16)


def shard_tokens(eidx, order):
    """Split expert-sorted tokens into NCORES shards, each spanning <= NSEG
    experts. Equal cuts when possible (perfect balance for typical routing);
    otherwise pair experts across cores (always <= 2 experts per core)."""
    shards = [order[c * BC:(c + 1) * BC] for c in range(NCORES)]
    if all(len(set(eidx[sl].tolist())) <= NSEG for sl in shards):
        return shards
    counts = np.bincount(eidx, minlength=E)
    by_size = np.argsort(counts)[::-1]
    shards = []
    for c in range(NCORES):
        pair = [by_size[c], by_size[2 * NCORES - 1 - c]]
        mask = np.isin(eidx[order], pair)
        shards.append(order[mask])
    return shards


# ----------------------------------------------------------------------------
# Numpy emulation (validates packing + dataflow)
# ----------------------------------------------------------------------------

def emulate_core(xb, w):
    import ml_dtypes

    res = np.zeros((NSEG, BC), np.float32)
    w1 = w["w1t"].astype(np.float32)
    for b in range(NB):
        xt = xb[b].astype(np.float32)  # [128, KT, NT]
        ps1 = np.zeros((NSEG * 16, NT), np.float32)
        for kt in range(KT):
            ps1 += w1[:, kt, :].T @ xt[:, kt, :]
        biased = ps1 + w["b1"]
        rsq = np.minimum(np.square(biased) * SQ_SCALE, 1.0)
        rlin = np.minimum(np.maximum(biased, 0.0), 1.0)
        ps2 = w["w2sq"].T @ rsq + w["w2lin"].T @ rlin
        l2x = np.minimum(np.maximum(ps2 + w["b2"], 0.0), 1.0)
        ps3 = w["w3p"].T @ l2x + w["rawsel"].T @ ps1 + w["ob"]
        res[:, b * NT:(b + 1) * NT] = ps3
    return res


# ----------------------------------------------------------------------------
# Bass program
# ----------------------------------------------------------------------------

def build_bass(nb=NB):
    import concourse.bacc as bacc
    import concourse.mybir as mybir
    import concourse.tile as tile
    from concourse.tile_rust import add_dep_helper

    f32 = mybir.dt.float32
    f32r = mybir.dt.float32r
    bf16 = mybir.dt.bfloat16
    AF = mybir.ActivationFunctionType
    OP = mybir.AluOpType

    nc = bacc.Bacc("TRN2", target_bir_lowering=False, debug=False)

    xb_d = nc.dram_tensor("xb", (nb, 128, KT, NT), bf16, kind="ExternalInput")
    w1t_d = nc.dram_tensor("w1t", (128, KT, NSEG * 16), bf16,
                           kind="ExternalInput")
    # bc: b1 | w2sq | w2lin | b2 | w3p | rawsel | ob  along free dim
    BCW = 1 + NSEG * 32 + NSEG * 32 + 1 + NSEG + NSEG + 1
    bc_d = nc.dram_tensor("bc", (128, BCW), f32r, kind="ExternalInput")
    res_d = nc.dram_tensor("res", (NSEG, nb * NT), f32, kind="ExternalOutput")

    _warm_n = int(os.environ.get("KERNEL_WARM", "24"))
    _warm_blk = int(os.environ.get("KERNEL_WARMBLK", "0"))
    _depth = int(os.environ.get("KERNEL_DMADEPTH", "6"))

    with tile.TileContext(nc) as tc:
        with (
            tc.tile_pool(name="consts", bufs=1) as consts,
            tc.tile_pool(name="xpool", bufs=8) as xpool,
            tc.tile_pool(name="acts", bufs=3) as acts,
            tc.tile_pool(name="ps1p", bufs=3, space="PSUM") as ps1p,
            tc.tile_pool(name="ps2p", bufs=2, space="PSUM") as ps2p,
            tc.tile_pool(name="ps3p", bufs=2, space="PSUM") as ps3p,
            tc.tile_pool(name="pswp", bufs=1, space="PSUM") as pswp,
        ):
            warm_sb = consts.tile([128, NT], bf16)
            warm_ps = pswp.tile([128, NT], f32, tag="warm")
            nc.vector.memset(warm_sb, 0.0)

            def warm(n):
                for _ in range(n):
                    nc.tensor.matmul(warm_ps, warm_sb[:, :128], warm_sb,
                                     start=True, stop=True)

            warm(_warm_n)

            _dma_chain = []

            def dma(out_ap, in_ap):
                inst = nc.sync.dma_start(out_ap, in_ap)
                _dma_chain.append(inst.ins)
                if _depth and len(_dma_chain) > _depth:
                    add_dep_helper(_dma_chain[-1], _dma_chain[-1 - _depth],
                                   reason="bound DMA in-flight window")
                return inst

            bc = consts.tile([128, BCW], f32r)
            dma(bc, bc_d[:])
            w1t = consts.tile([128, KT, NSEG * 16], bf16)
            dma(w1t, w1t_d[:])
            b1 = bc[0:NSEG * 16, 0:1]
            w2sq = bc[0:NSEG * 16, 1:1 + NSEG * 32]
            w2lin = bc[0:NSEG * 16, 1 + NSEG * 32:1 + NSEG * 64]
            b2 = bc[0:NSEG * 32, 1 + NSEG * 64:2 + NSEG * 64]
            w3p = bc[0:NSEG * 32, 2 + NSEG * 64:2 + NSEG * 65]
            rawsel = bc[0:NSEG * 16, 2 + NSEG * 65:2 + NSEG * 66]
            ob = bc[0:NSEG, 2 + NSEG * 66:3 + NSEG * 66].bitcast(f32)

            resbuf = consts.tile([NSEG, nb * NT], f32)

            NXC = int(os.environ.get("KERNEL_XCHUNKS", "4"))
            KPC = KT // NXC

            def load_block(b):
                tiles = []
                for c in range(NXC):
                    xc = xpool.tile([128, KPC, NT], bf16, tag="xt")
                    dma(xc, xb_d[b, :, KPC * c:KPC * (c + 1), :])
                    tiles.append(xc)
                return tiles

            xtc0 = load_block(0)

            def do_l2l3(b, rsq, rlin, raw):
                # PE work for block b's tail; deps (rsq/rlin) are a full
                # block old, so these matmuls never stall the PE stream
                ps2 = ps2p.tile([NSEG * 32, NT], f32, tag="ps2")
                nc.tensor.matmul(ps2, w2sq, rsq, start=True, stop=False)
                nc.tensor.matmul(ps2, w2lin, rlin, start=False, stop=True)
                l2x = acts.tile([NSEG * 32, NT], f32r, tag="l2x")
                nc.scalar.activation(l2x, ps2, AF.Relu, bias=b2)
                nc.vector.tensor_scalar_min(l2x, l2x, 1.0)

                ps3 = ps3p.tile([NSEG, NT], f32, tag="ps3")
                nc.tensor.matmul(ps3, w3p, l2x, start=True, stop=False)
                nc.tensor.matmul(ps3, rawsel, raw, start=False, stop=True)
                nc.scalar.copy(resbuf[:, b * NT:(b + 1) * NT], ps3)

            xtc_next = xtc0
            pending = None
            for b in range(nb):
                xtc = xtc_next
                if b + 1 < nb:
                    xtc_next = load_block(b + 1)

                if _warm_blk and b > 0:
                    warm(_warm_blk)

                ps1 = ps1p.tile([NSEG * 16, NT], f32, tag="ps1")
                for kt in range(KT):
                    nc.tensor.matmul(
                        ps1,
                        w1t[:, kt, :],
                        xtc[kt // KPC][:, kt % KPC, :],
                        start=(kt == 0), stop=(kt == KT - 1),
                    )
                if pending is not None:
                    do_l2l3(*pending)
                rsq = acts.tile([NSEG * 16, NT], f32r, tag="rsq")
                rlin = acts.tile([NSEG * 16, NT], f32r, tag="rlin")
                raw = acts.tile([NSEG * 16, NT], f32r, tag="raw")
                nc.scalar.activation(rsq, ps1, AF.Square, bias=b1)
                nc.vector.tensor_scalar(rsq, rsq, SQ_SCALE, 1.0,
                                        OP.mult, OP.min)
                nc.scalar.activation(rlin, ps1, AF.Relu, bias=b1)
                nc.vector.tensor_scalar_min(rlin, rlin, 1.0)
                nc.scalar.copy(raw, ps1)
                pending = (b, rsq, rlin, raw)

            do_l2l3(*pending)
            nc.sync.dma_start(res_d[:], resbuf)

    nc.compile()
    return nc


# ----------------------------------------------------------------------------
# Entry point
# ----------------------------------------------------------------------------

def kernel(**inputs):
    from concourse.bass_utils import run_bass_kernel_spmd

    x = np.asarray(inputs["expert_input"], np.float32)
    eidx, order = route_tokens(inputs["router_input"], inputs["router_w"],
                               inputs["router_b"])
    l1_w = np.asarray(inputs["l1_w"], np.float32)
    l1_b = np.asarray(inputs["l1_b"], np.float32)
    l2_w = np.asarray(inputs["l2_w"], np.float32)
    l2_b = np.asarray(inputs["l2_b"], np.float32)
    out_w = np.asarray(inputs["out_w"], np.float32)
    out_b = np.asarray(inputs["out_b"], np.float32)

    shards = shard_tokens(eidx, order)
    nb = max(1, -(-max(len(sl) for sl in shards) // NT))
    in_maps = []
    core_seg = []  # (order_slice, seg_of_token)
    core_ob = []
    for c in range(NCORES):
        sl = shards[c]
        ex = eidx[sl]
        experts = sorted(set(ex.tolist()))
        assert len(experts) <= NSEG, f"core {c} spans {len(experts)} experts"
        seg = np.searchsorted(np.asarray(experts), ex)
        w = pack_core_weights(experts, l1_w, l1_b, l2_w, l2_b, out_w, out_b)
        bc = np.zeros((128, 1 + NSEG * 32 + NSEG * 32 + 1 + NSEG + NSEG + 1), np.float32)
        col = 0
        bc[0:NSEG * 16, col:col + 1] = w["b1"]; col += 1
        bc[0:NSEG * 16, col:col + NSEG * 32] = w["w2sq"]; col += NSEG * 32
        bc[0:NSEG * 16, col:col + NSEG * 32] = w["w2lin"]; col += NSEG * 32
        bc[0:NSEG * 32, col:col + 1] = w["b2"]; col += 1
        bc[0:NSEG * 32, col:col + NSEG] = w["w3p"]; col += NSEG
        bc[0:NSEG * 16, col:col + NSEG] = w["rawsel"]; col += NSEG
        bc[0:NSEG, col:col + 1] = w["ob"]; col += 1
        in_maps.append({
            "xb": pack_x_shard_bf16(x[sl], nb),
            "w1t": w["w1t"],
            "bc": bc,
        })
        core_seg.append((sl, seg))
        core_ob.append(w["ob"][:, 0].copy())

    nc = build_bass(nb)
    trace = bool(int(os.environ.get("KERNEL_TRACE", "0")))
    out = run_bass_kernel_spmd(nc, in_maps, core_ids=list(range(NCORES)),
                               trace=trace)
    if trace:
        kernel.last_exec_time_ns = out.exec_time_ns
        kernel.last_trace = out.instructions_and_trace

    result = np.zeros((B, 1), np.float32)
    for c in range(NCORES):
        sl, seg = core_seg[c]
        res = out.results[c]["res"]  # [NSEG, nb*NT]
        obh = core_ob[c]
        result[sl, 0] = res[seg, np.arange(len(sl))] + obh[seg]
    return result
